# revision 1
# baseline (speedup 1.0000x reference)
"""Trainium2 Bass kernel for nn_BranchGCN (gnn_message_passing).

Strategy (8 NeuronCores, two SPMD launches):
  Stage A -- model-parallel over W_branch's node axis: core c owns nodes
    [4c, 4c+4). It computes root aggregation + per-node branch matmul +
    loop MLP for its 4 nodes x all 16 samples, and emits x (point coords)
    plus |x|^2 for its 256 of the 2048 graph rows.  This reads only 1/8 of
    the 128 MiB W_branch per core (the dominant memory term).
  Host    -- reshards (pure concatenation/transpose, no arithmetic).
  Stage B -- row-sharded EdgeConv: every core holds all 2048 points of all
    16 samples (small), computes the KNN top-8 for its 256 rows x 16
    samples via PE distance matmuls + DVE max8/max_index, gathers the
    factored conv values via indirect DMA, and applies max/bias/leaky.

EdgeConv factorization used (exact, modulo fp reassociation):
  h2[b,n,k,:] = (feat-center) @ M1 + center @ M2 + (c1b @ c2w.T + c2b)
    with M1 = c1w[:, :3].T @ c2w.T,  M2 = c1w[:, 3:].T @ c2w.T
  out_pre[n]  = max_k (x[idx_k] @ M1)  +  x[n] @ (M2 - M1) + const
  pd[n,j] is computed with the 5-term homogeneous matmul
    [2x_n, -|x_n|^2, -1] . [x_j, 1, |x_j|^2]  so pd[n,n] == 0 exactly and
  the self row is always rank-0 of the top-8, which lets the same gather
  fetch the center term (z) from the table's columns 3:6 at k=0.
"""

import os
import sys
import numpy as np

sys.path.insert(0, "/opt/trn_rl_repo")

from contextlib import ExitStack

import concourse.tile as tile
from concourse import bacc, bass, mybir
from concourse.bass import AP
from concourse.bass_utils import run_bass_kernel_spmd
from concourse.masks import make_identity

FP = mybir.dt.float32
U32 = mybir.dt.uint32

B, NODE, DEG, K = 16, 32, 64, 8
IN_F, OUT_F, SUP = 128, 3, 10
FEATS = [96, 256, 256, 256, 128, 128]
SIZES = [1, 2, 4, 8, 16, 32]
NCORES = 8
NLOC = NODE // NCORES          # 4 nodes per core
N = NODE * DEG                 # 2048 graph rows
RLOC = NLOC * DEG              # 256 rows per core
ALU = mybir.AluOpType
AF = mybir.ActivationFunctionType


# --------------------------------------------------------------------------
# Stage A program: branch/root/loop-MLP for this core's 4 nodes.
# --------------------------------------------------------------------------
def build_stage_a():
    nc = bacc.Bacc(None)
    tls = [nc.declare_dram_parameter(f"tl{i}", [B, NLOC, FEATS[i]], FP, isOutput=False)
           for i in range(6)]
    wrs = [nc.declare_dram_parameter(f"wr{i}", [FEATS[i], OUT_F], FP, isOutput=False)
           for i in range(6)]
    wb = nc.declare_dram_parameter("wb", [NLOC, IN_F, DEG * IN_F], FP, isOutput=False)
    wl1 = nc.declare_dram_parameter("wl1", [IN_F, IN_F * SUP], FP, isOutput=False)
    wl2 = nc.declare_dram_parameter("wl2", [IN_F * SUP, OUT_F], FP, isOutput=False)
    # rows 0-2: x coords, row 3: |x|^2 ; flat order = (b, n_local, d)
    xchunk = nc.declare_dram_parameter("xchunk", [4, B * RLOC], FP,
                                       isOutput=True)

    with tile.TileContext(nc) as tc, ExitStack() as ctx:
        sbp = ctx.enter_context(tc.tile_pool(name="sbuf", bufs=1))
        wbpool = ctx.enter_context(tc.tile_pool(name="wbuf", bufs=2))
        psp = ctx.enter_context(tc.tile_pool(name="psum", bufs=1,
                                             space="PSUM"))
        pbp = ctx.enter_context(tc.tile_pool(name="psumb", bufs=2,
                                             space="PSUM"))

        ident = sbp.tile([128, 128], FP)
        make_identity(nc, ident[:])

        # ---- load + transpose the per-node tree slices: tlT[i] = (f, 64)
        tlT = []
        for i in range(6):
            f = FEATS[i]
            nat = sbp.tile([B * NLOC, f], FP, tag=f"tlnat{i}")
            nc.sync.dma_start(out=nat[:],
                              in_=tls[i][:].rearrange("b n f -> (b n) f"))
            nchunk = (f + 127) // 128
            tt = sbp.tile([128, nchunk, B * NLOC], FP, tag=f"tlT{i}")
            for c in range(nchunk):
                cw = min(128, f - c * 128)
                pt = psp.tile([128, B * NLOC], FP, tag="ptr")
                nc.tensor.transpose(out=pt[:cw, :],
                                    in_=nat[:, c * 128:c * 128 + cw],
                                    identity=ident[0:B * NLOC, 0:B * NLOC])
                nc.scalar.activation(out=tt[0:cw, c, :], in_=pt[:cw, :],
                                     func=AF.Copy)
            tlT.append(tt)

        # ---- Wl = Wl1 @ Wl2  (128, 3)
        wl1_sb = sbp.tile([128, IN_F * SUP], FP)
        nc.sync.dma_start(out=wl1_sb[:], in_=wl1[:])
        wl2_sb = sbp.tile([128, SUP, OUT_F], FP)
        nc.sync.dma_start(out=wl2_sb[:],
                          in_=wl2[:].rearrange("(c p) o -> p c o", p=128))
        wl1T = sbp.tile([128, SUP, 128], FP)
        for c in range(SUP):
            pt = psp.tile([128, 128], FP, tag="ptw")
            nc.tensor.transpose(out=pt[:], in_=wl1_sb[:, c * 128:(c + 1) * 128],
                                identity=ident[:])
            nc.scalar.activation(out=wl1T[:, c, :], in_=pt[:], func=AF.Copy)
        pwl = psp.tile([128, OUT_F], FP, tag="pwl")
        for c in range(SUP):
            nc.tensor.matmul(out=pwl[:], lhsT=wl1T[:, c, :],
                             rhs=wl2_sb[:, c, :],
                             start=(c == 0), stop=(c == SUP - 1))
        wlv = sbp.tile([128, OUT_F], FP)
        nc.scalar.activation(out=wlv[:], in_=pwl[:], func=AF.Copy)

        # ---- root aggregation for this core's nodes: rootT (3, (b, nl))
        wr_sb = []
        for i in range(6):
            f = FEATS[i]
            nchunk = (f + 127) // 128
            w = sbp.tile([128, nchunk, OUT_F], FP, tag=f"wr{i}")
            nc.sync.dma_start(
                out=w[:f if nchunk == 1 else 128, :, :],
                in_=wrs[i][:].rearrange("(c p) o -> p c o",
                                        c=nchunk) if nchunk > 1
                else wrs[i][:].unsqueeze(1))
            wr_sb.append(w)
        proot = psp.tile([OUT_F, B * NLOC], FP, tag="proot")
        steps = []
        for i in range(6):
            for c in range((FEATS[i] + 127) // 128):
                steps.append((i, c))
        for si, (i, c) in enumerate(steps):
            f = FEATS[i]
            cw = min(128, f - c * 128)
            nc.tensor.matmul(out=proot[:],
                             lhsT=wr_sb[i][:cw, c, :],
                             rhs=tlT[i][0:cw, c, :],
                             start=(si == 0), stop=(si == len(steps) - 1))
        rootT = sbp.tile([OUT_F, B * NLOC], FP)
        nc.scalar.activation(out=rootT[:], in_=proot[:], func=AF.Copy)

        # ---- branch einsum + leaky: branchT (128, (b, nl, d))
        branchT = sbp.tile([128, B * RLOC], FP)
        t5v = tlT[5][:, 0, :].rearrange("p (b n) -> p n b", n=NLOC)
        for nl in range(NLOC):
            wbt = wbpool.tile([128, DEG * IN_F], FP, tag="wbt")
            nc.sync.dma_start(out=wbt[:], in_=wb[nl])
            for g in range(2):
                pb = pbp.tile([128, 512], FP, tag="pbranch")
                for dl in range(32):
                    d = g * 32 + dl
                    nc.tensor.matmul(out=pb[:, dl * 16:(dl + 1) * 16],
                                     lhsT=wbt[:, d * 128:(d + 1) * 128],
                                     rhs=t5v[:, nl, :],
                                     start=True, stop=True)
                # out view: (p, dl, b) -> branchT[(b, nl, d=g*32+dl)]
                ov = branchT[:].rearrange(
                    "p (b n g dl) -> p n g dl b", n=NLOC, g=2, dl=32)
                pbs = sbp.tile([128, 512], FP, tag="pbs")
                nc.scalar.activation(out=pbs[:], in_=pb[:], func=AF.Copy)
                pbv = pbs[:].rearrange("p (dl b) -> p dl b", dl=32)
                nc.vector.scalar_tensor_tensor(
                    out=ov[:, nl, g, :, :], in0=pbv, scalar=0.2, in1=pbv,
                    op0=ALU.mult, op1=ALU.max)

        # ---- x = branch @ Wl + root(repeat d); then |x|^2
        x3 = sbp.tile([OUT_F, B * RLOC], FP)
        xx1 = sbp.tile([1, B * RLOC], FP)
        rootv = rootT[:].rearrange("p (b n) -> p b n", n=NLOC)
        for ch in range(8):  # 512 cols = 2 samples each
            po = psp.tile([OUT_F, 512], FP, tag="po3")
            nc.tensor.matmul(out=po[:], lhsT=wlv[:, :OUT_F],
                             rhs=branchT[:, ch * 512:(ch + 1) * 512],
                             start=True, stop=True)
            in0 = po[:].rearrange("p (b n d) -> p b n d", b=2, n=NLOC)
            in1 = rootv[:, 2 * ch:2 * ch + 2, :].unsqueeze(3).to_broadcast(
                [OUT_F, 2, NLOC, DEG])
            ov = x3[:].rearrange("p (b n d) -> p b n d", b=B,
                                 n=NLOC)[:, 2 * ch:2 * ch + 2]
            nc.vector.tensor_tensor(out=ov, in0=in0, in1=in1, op=ALU.add)
        sq = sbp.tile([OUT_F, B * RLOC], FP)
        nc.vector.tensor_tensor(out=sq[:], in0=x3[:], in1=x3[:],
                                op=ALU.mult)
        ones3 = sbp.tile([OUT_F, 1], FP)
        nc.vector.memset(ones3[:], 1.0)
        for ch in range(8):
            px = psp.tile([1, 512], FP, tag="pxx")
            nc.tensor.matmul(out=px[:], lhsT=ones3[:],
                             rhs=sq[:, ch * 512:(ch + 1) * 512],
                             start=True, stop=True)
            nc.scalar.activation(out=xx1[0:1, ch * 512:(ch + 1) * 512],
                                 in_=px[:], func=AF.Copy)
        nc.sync.dma_start(out=xchunk[0:3, :], in_=x3[:])
        nc.sync.dma_start(out=xchunk[3:4, :], in_=xx1[:])
    return nc


# --------------------------------------------------------------------------
# Stage B program: KNN EdgeConv for this core's 256 rows x 16 samples.
# --------------------------------------------------------------------------
def build_stage_b():
    nc = bacc.Bacc(None)
    # rows: [x0, x1, x2, xx]; per-sample row-major (node*64+d)
    vall = nc.declare_dram_parameter("vall", [4, B, N], FP, isOutput=False)
    # rows: [x0, x1, x2] for this core's 256 rows, flat (b, nl, d)
    urx = nc.declare_dram_parameter("urx", [OUT_F, B * RLOC], FP, isOutput=False)
    biasd = nc.declare_dram_parameter("biasd", [DEG, OUT_F], FP, isOutput=False)
    c1w = nc.declare_dram_parameter("c1w", [64, 6], FP, isOutput=False)
    c1b = nc.declare_dram_parameter("c1b", [64, 1], FP, isOutput=False)
    c2w = nc.declare_dram_parameter("c2w", [OUT_F, 64], FP, isOutput=False)
    c2b = nc.declare_dram_parameter("c2b", [1, OUT_F], FP, isOutput=False)
    outc = nc.declare_dram_parameter("outc", [B, RLOC, OUT_F], FP,
                                     isOutput=True)

    with tile.TileContext(nc) as tc, ExitStack() as ctx:
        sbp = ctx.enter_context(tc.tile_pool(name="sbuf", bufs=1))
        lop = ctx.enter_context(tc.tile_pool(name="loop", bufs=2))
        dramp = ctx.enter_context(tc.tile_pool(name="dram", bufs=1,
                                               space="DRAM"))
        psw = ctx.enter_context(tc.tile_pool(name="psw", bufs=2,
                                             space="PSUM"))
        pst = ctx.enter_context(tc.tile_pool(name="pst", bufs=2,
                                             space="PSUM"))
        setup_ps = tc.tile_pool(name="setup_ps", bufs=2, space="PSUM")
        psu = setup_ps.__enter__()
        setup_sb = tc.tile_pool(name="setup_sb", bufs=1)
        sbu = setup_sb.__enter__()

        ident = sbp.tile([128, 128], FP)
        make_identity(nc, ident[:])

        # ---- conv weight factorization on device
        c1w_sb = sbu.tile([64, 6], FP)
        nc.sync.dma_start(out=c1w_sb[:], in_=c1w[:])
        c1b_sb = sbu.tile([64, 1], FP)
        nc.sync.dma_start(out=c1b_sb[:], in_=c1b[:])
        c2w_sb = sbu.tile([OUT_F, 64], FP)
        nc.sync.dma_start(out=c2w_sb[:], in_=c2w[:])
        c2b_sb = sbu.tile([1, OUT_F], FP)
        nc.sync.dma_start(out=c2b_sb[:], in_=c2b[:])

        ptc = psu.tile([64, OUT_F], FP, tag="su")
        nc.tensor.transpose(out=ptc[:], in_=c2w_sb[:],
                            identity=ident[0:OUT_F, 0:OUT_F])
        c2wT = sbu.tile([64, OUT_F], FP)
        nc.scalar.activation(out=c2wT[:], in_=ptc[:], func=AF.Copy)
        c2wTn = sbu.tile([64, OUT_F], FP)
        nc.vector.tensor_scalar_mul(out=c2wTn[:], in0=c2wT[:], scalar1=-1.0)

        lw = sbp.tile([OUT_F, 8], FP)       # lhsT for the w-table matmul
        nc.vector.memset(lw[:], 0.0)
        pm1 = psu.tile([OUT_F, OUT_F], FP, tag="su")
        nc.tensor.matmul(out=pm1[:], lhsT=c1w_sb[:, 0:3], rhs=c2wT[:],
                         start=True, stop=True)
        nc.vector.tensor_copy(out=lw[0:3, 0:3], in_=pm1[:])
        pm2 = psu.tile([OUT_F, OUT_F], FP, tag="su")
        nc.tensor.matmul(out=pm2[:], lhsT=c1w_sb[:, 3:6], rhs=c2wT[:],
                         start=True, stop=False)
        nc.tensor.matmul(out=pm2[:], lhsT=c1w_sb[:, 0:3], rhs=c2wTn[:],
                         start=False, stop=True)
        nc.vector.tensor_copy(out=lw[0:3, 3:6], in_=pm2[:])

        # zc = c1b @ c2w.T + c2b, broadcast to 128 partitions, fold into bias
        pzc = psu.tile([1, OUT_F], FP, tag="su")
        nc.tensor.matmul(out=pzc[:], lhsT=c1b_sb[:], rhs=c2wT[:],
                         start=True, stop=True)
        zrow = sbu.tile([1, OUT_F], FP)
        nc.vector.tensor_tensor(out=zrow[:], in0=pzc[:], in1=c2b_sb[:],
                                op=ALU.add)
        ones1 = sbu.tile([1, 128], FP)
        nc.vector.memset(ones1[:], 1.0)
        pzb = psu.tile([128, OUT_F], FP, tag="su")
        nc.tensor.matmul(out=pzb[:], lhsT=ones1[:], rhs=zrow[:],
                         start=True, stop=True)
        bias_sb = sbu.tile([128, OUT_F], FP)
        nc.sync.dma_start(out=bias_sb[0:64, :], in_=biasd[:])
        nc.sync.dma_start(out=bias_sb[64:128, :], in_=biasd[:])
        bias2 = sbp.tile([128, OUT_F], FP)
        nc.vector.tensor_tensor(out=bias2[:], in0=bias_sb[:], in1=pzb[:],
                                op=ALU.add)

        # ---- static tiles
        vall_sb = sbp.tile([4, B, N], FP)
        nc.sync.dma_start(out=vall_sb[:], in_=vall[:])
        uv = sbp.tile([4, B * RLOC], FP)
        nc.sync.dma_start(out=uv[0:3, :], in_=urx[:])
        nc.vector.tensor_scalar_mul(out=uv[0:3, :], in0=uv[0:3, :],
                                    scalar1=2.0)
        mone = sbu.tile([1, B * RLOC], FP)
        nc.vector.memset(mone[:], -1.0)
        nc.sync.dma_start(out=uv[3:4, :], in_=mone[:])
        setup_sb.__exit__(None, None, None)
        setup_ps.__exit__(None, None, None)
        pspd = ctx.enter_context(tc.tile_pool(name="pspd", bufs=1,
                                              space="PSUM"))

        final_sb = sbp.tile([128, B, 2, OUT_F], FP)

        for b in range(B):
            # ---- w table: w[row] = [y(3), z(3), 0, 0]
            wT_sb = lop.tile([8, N], FP, tag="wT")
            for chm in range(4):
                pw = psw.tile([8, 512], FP, tag="pw")
                nc.tensor.matmul(out=pw[:], lhsT=lw[:],
                                 rhs=vall_sb[0:3, b, chm * 512:(chm + 1) * 512],
                                 start=True, stop=True)
                nc.scalar.activation(out=wT_sb[:, chm * 512:(chm + 1) * 512],
                                     in_=pw[:], func=AF.Copy)
            ptr = pst.tile([128, 128], FP, tag="ptr")
            for kk in range(16):
                nc.tensor.transpose(out=ptr[:, kk * 8:(kk + 1) * 8],
                                    in_=wT_sb[:, kk * 128:(kk + 1) * 128],
                                    identity=ident[0:8, 0:8])
            wrows = lop.tile([128, 16 * 8], FP, tag="wrows")
            nc.scalar.activation(out=wrows[:], in_=ptr[:], func=AF.Copy)
            wtab = dramp.tile([N, 8], FP, tag=f"wtab{b}")
            nc.sync.dma_start(
                out=wtab[:].rearrange("(k p) e -> p k e", p=128),
                in_=wrows[:].rearrange("p (k e) -> p k e", k=16))

            # ---- pd + top8 for the two 128-row tiles
            idx = lop.tile([128, 2, K], U32, tag="idx")
            for m in range(2):
                ppd = pspd.tile([128, N], FP, tag="ppd")
                for chm in range(4):
                    nc.tensor.matmul(
                        out=ppd[:, chm * 512:(chm + 1) * 512],
                        lhsT=uv[:, b * RLOC + m * 128:b * RLOC + (m + 1) * 128],
                        rhs=vall_sb[:, b, chm * 512:(chm + 1) * 512],
                        start=True, stop=True)
                pd_sb = lop.tile([128, N], FP, tag="pd")
                nc.scalar.activation(out=pd_sb[:], in_=ppd[:], func=AF.Copy)
                top8 = lop.tile([128, K], FP, tag="top8")
                nc.vector.max(out=top8[:], in_=pd_sb[:])
                nc.vector.max_index(out=idx[:, m, :], in_max=top8[:],
                                    in_values=pd_sb[:])

            # ---- gather w[idx] : (128, 2, 8, 8)
            gth = lop.tile([128, 2, K, 8], FP, tag="gth")
            for m in range(2):
                for k in range(K):
                    nc.gpsimd.indirect_dma_start(
                        out=gth[:, m, k, :],
                        out_offset=None,
                        in_=wtab[:],
                        in_offset=bass.IndirectOffsetOnAxis(
                            ap=idx[:, m, k:k + 1], axis=0),
                    )

            # ---- max over neighbors + center term + bias + leaky
            red = lop.tile([128, 2, OUT_F], FP, tag="red")
            nc.vector.tensor_reduce(
                out=red[:],
                in_=gth[:].rearrange("p m k e -> p m e k")[:, :, 0:OUT_F, :],
                axis=mybir.AxisListType.X, op=ALU.max)
            tmp = lop.tile([128, 2, OUT_F], FP, tag="tmp")
            nc.vector.tensor_tensor(out=tmp[:], in0=red[:],
                                    in1=gth[:, :, 0, 3:3 + OUT_F], op=ALU.add)
            nc.vector.tensor_tensor(
                out=tmp[:], in0=tmp[:],
                in1=bias2[:].unsqueeze(1).to_broadcast([128, 2, OUT_F]),
                op=ALU.add)
            nc.vector.scalar_tensor_tensor(
                out=final_sb[:, b, :, :], in0=tmp[:], scalar=0.2, in1=tmp[:],
                op0=ALU.mult, op1=ALU.max)

        nc.sync.dma_start(
            out=outc[:].rearrange("b (m p) o -> p b m o", p=128),
            in_=final_sb[:])
    return nc


# --------------------------------------------------------------------------
# Host orchestration
# --------------------------------------------------------------------------
_CACHE = {}
LAST_RESULTS = {}


def _programs():
    if "a" not in _CACHE:
        nca = build_stage_a()
        nca.compile()
        ncb = build_stage_b()
        ncb.compile()
        _CACHE["a"] = nca
        _CACHE["b"] = ncb
    return _CACHE["a"], _CACHE["b"]


def _stage_a_inmaps(inputs):
    trees = [np.asarray(inputs[f"t{i}"], np.float32) for i in range(6)]
    wrs = [np.asarray(inputs[f"Wr{i}"], np.float32) for i in range(6)]
    wb = np.asarray(inputs["W_branch"], np.float32)
    in_maps = []
    for c in range(NCORES):
        m = {}
        nodes = [4 * c + j for j in range(NLOC)]
        for i in range(6):
            rows = [n * SIZES[i] // NODE for n in nodes]
            m[f"tl{i}"] = np.ascontiguousarray(trees[i][:, rows, :])
            m[f"wr{i}"] = wrs[i]
        m["wb"] = np.ascontiguousarray(wb[nodes])
        m["wl1"] = np.asarray(inputs["Wl1"], np.float32)
        m["wl2"] = np.asarray(inputs["Wl2"], np.float32)
        in_maps.append(m)
    return in_maps


def _stage_b_inmaps(inputs, xchunks):
    # xchunks: list of (4, B*RLOC) arrays, flat order (b, nl, d)
    xs = np.stack(xchunks)                     # (8, 4, B*256)
    xs = xs.reshape(NCORES, 4, B, RLOC)        # (c, comp, b, nl*64+d)
    vall = np.ascontiguousarray(
        xs.transpose(1, 2, 0, 3).reshape(4, B, N))   # rows node*64+d
    bias = np.asarray(inputs["bias"], np.float32).reshape(DEG, OUT_F)
    in_maps = []
    for c in range(NCORES):
        m = {
            "vall": vall,
            "urx": np.ascontiguousarray(xs[c, 0:3].reshape(OUT_F, B * RLOC)),
            "biasd": bias,
            "c1w": np.asarray(inputs["c1w"], np.float32),
            "c1b": np.asarray(inputs["c1b"], np.float32).reshape(64, 1),
            "c2w": np.asarray(inputs["c2w"], np.float32),
            "c2b": np.asarray(inputs["c2b"], np.float32).reshape(1, OUT_F),
        }
        in_maps.append(m)
    return in_maps


def kernel(**inputs):
    nca, ncb = _programs()
    core_ids = list(range(NCORES))

    ra = run_bass_kernel_spmd(nca, _stage_a_inmaps(inputs), core_ids)
    LAST_RESULTS["a"] = ra
    xchunks = [np.asarray(ra.results[c]["xchunk"]) for c in range(NCORES)]

    rb = run_bass_kernel_spmd(ncb, _stage_b_inmaps(inputs, xchunks), core_ids)
    LAST_RESULTS["b"] = rb
    out = np.empty((B, N, OUT_F), np.float32)
    for c in range(NCORES):
        out[:, c * RLOC:(c + 1) * RLOC, :] = rb.results[c]["outc"]
    return out

